# revision 8
# baseline (speedup 1.0000x reference)
"""Trainium2 Bass kernel for nn_CrossEntropyLoss_22419729285187.

Computes  -sum_{matched, non-BG true rows} dot(y_true[i,1:], y_pred[rank_i]) / count
sharded over 8 NeuronCores.

Strategy (per sharding hint): the host performs the cheap key join
(encode + searchsorted + cumsum) and compacts to the contributing
(matched AND non-background) row pairs — the r-th matched true row
pairs positionally with y_pred_features[r], so both sides are plain
host gathers.  The pairs are cast to fp16 (~3e-4 relative
perturbation, far under the 2e-2 gate) and row-sharded across the 8
cores.  Per core the two shards are interleaved into ONE stream
([tile][partition][yt-line | yp-line]) so each tile is a single DMA
with 8KB contiguous per-partition lines — fewer descriptors and
completion-semaphore lanes, so tiles arrive in strict FIFO order right
behind the bytes.  The DVE runs one fused multiply-reduce
(scalar_tensor_tensor) per tile, accumulating per-tile partial sums in
fp32; a descending mini-tail keeps the post-stream DVE work ~0.3us.
The non-BG count k and the final -num/k division are host-side scalar
work.
"""

import os
import sys

for _p in ("/opt/trn_rl_repo", "/root/.axon_site/_ro/trn_rl_repo"):
    if os.path.isdir(_p) and _p not in sys.path:
        sys.path.append(_p)

import numpy as np

N_CORES = 8

PARTS = 128
G = 64  # rows per partition per tile (main segment)

_compiled = {}
_last_results = None


def _encode(idx):
    idx = idx.astype(np.int64)
    return ((idx[:, 0] * 1024 + idx[:, 1]) * 1024 + idx[:, 2]) * 1024 + idx[:, 3]


def _plan_segments(rows):
    """Tile plan for one core: list of g (rows-per-partition) per tile."""
    units = -(-rows // PARTS)  # 128-row units
    nbig = units // G
    rem = units - nbig * G
    gs = [G] * nbig
    if rem > 8:
        gs += [rem - 8, 8]
    elif rem > 0:
        gs += [rem]
    elif nbig > 1:
        gs = [G] * (nbig - 1) + [G - 8, 8]
    return gs


def _build_program(gs, c_pred):
    """Build + schedule the SPMD Tile program for one core shard.

    gs: rows-per-partition for each tile. The single input stream is
    laid out [tile][partition][g*c yt | g*c yp] fp16, contiguous.
    """
    from concourse import bacc
    import concourse.mybir as mybir
    from concourse.tile import TileContext

    f16 = mybir.dt.float16
    f32 = mybir.dt.float32
    total = sum(2 * g * c_pred * PARTS for g in gs)
    n_tiles = len(gs)

    nc = bacc.Bacc("TRN2", target_bir_lowering=False, debug=False,
                   num_devices=N_CORES)
    x_d = nc.dram_tensor("x", [total], f16, kind="ExternalInput")
    out_d = nc.dram_tensor("partials", [PARTS, n_tiles], f32,
                           kind="ExternalOutput")

    with TileContext(nc) as tc:
        with tc.tile_pool(name="acc", bufs=1) as accp:
            red_all = accp.tile([PARTS, n_tiles], f32)
            # One distinct buffer per tile: all DMAs are issued upfront
            # and arrive in FIFO order with no buffer-reuse stalls.
            # Per tile: DVE multiplies (tensor_tensor hits the 2x 16-bit
            # mode), the otherwise-idle ACT engine reduces via
            # activation(Copy) + accum. Both run far under the stream
            # cadence, so neither instruction chain lags the DMAs.
            with tc.tile_pool(name="io", bufs=n_tiles) as pool, \
                 tc.tile_pool(name="scrp", bufs=3) as scrp, \
                 tc.tile_pool(name="dump", bufs=2) as dump:
                off = 0
                for ti, g in enumerate(gs):
                    w = 2 * g * c_pred
                    view = x_d.ap()[off:off + w * PARTS].rearrange(
                        "(p w) -> p w", p=PARTS)
                    off += w * PARTS
                    xt = pool.tile([PARTS, w], f16, tag="x")
                    nc.sync.dma_start(out=xt[:], in_=view)
                    scr = scrp.tile([PARTS, g * c_pred], f16, tag="scr")
                    nc.vector.tensor_tensor(
                        out=scr[:], in0=xt[:, :g * c_pred],
                        in1=xt[:, g * c_pred:], op=mybir.AluOpType.mult)
                    dmp = dump.tile([PARTS, g * c_pred], f16, tag="d")
                    # red_all[:, ti] = sum_{g,c} scr
                    nc.scalar.activation(
                        out=dmp[:], in_=scr[:],
                        func=mybir.ActivationFunctionType.Copy,
                        accum_out=red_all[:, ti:ti + 1])
            nc.scalar.dma_start(out=out_d[:], in_=red_all[:])
    nc.compile()
    return nc


def kernel(y_true_features, y_true_indices, y_pred_features, y_pred_indices):
    global _last_results
    from concourse.bass_utils import run_bass_kernel_spmd

    yt = np.asarray(y_true_features, dtype=np.float32)
    yp = np.asarray(y_pred_features, dtype=np.float32)
    n, c1 = yt.shape
    m, c = yp.shape

    # ---- host-side key join (cheap integer work) ----
    kt = _encode(np.asarray(y_true_indices))
    kp = _encode(np.asarray(y_pred_indices))
    kps = np.sort(kp)
    pos = np.clip(np.searchsorted(kps, kt), 0, m - 1)
    matched = kps[pos] == kt
    # Only matched, non-background true rows contribute. The r-th
    # matched true row (row order) pairs with y_pred_features[r]
    # positionally (rank = cumsum(matched)-1 is sequential over matched
    # rows), so compacting to the contributing pairs is two host
    # gathers; k is their count.
    midx = np.flatnonzero(matched)
    keep = np.flatnonzero(yt[midx, 0] != 1.0)   # positions within matched
    k = keep.size
    yt_cmp = yt[midx[keep], 1:].astype(np.float16)   # [k, c]
    yp_cmp = yp[keep].astype(np.float16)             # [k, c]

    # ---- shard the k contributing pairs across cores ----
    rows = -(-k // N_CORES)
    gs = _plan_segments(rows)
    r_pad = PARTS * sum(gs)

    key = (tuple(gs), c)
    if key not in _compiled:
        _compiled[key] = _build_program(gs, c)
    nc = _compiled[key]

    total = 2 * r_pad * c
    in_maps = []
    for i in range(N_CORES):
        lo, hi = i * rows, min((i + 1) * rows, k)
        nr = max(hi - lo, 0)
        a = np.zeros((r_pad, c), dtype=np.float16)
        a[:nr] = yt_cmp[lo:hi]
        b = np.zeros((r_pad, c), dtype=np.float16)
        b[:nr] = yp_cmp[lo:hi]
        # interleave per tile: [tile][partition][g*c of a | g*c of b]
        x = np.empty(total, dtype=np.float16)
        off = 0
        r0 = 0
        for g in gs:
            trows = PARTS * g
            w = 2 * g * c
            blk = x[off:off + trows * 2 * c].reshape(PARTS, w)
            blk[:, :g * c] = a[r0:r0 + trows].reshape(PARTS, g * c)
            blk[:, g * c:] = b[r0:r0 + trows].reshape(PARTS, g * c)
            off += trows * 2 * c
            r0 += trows
        in_maps.append({"x": x})

    res = run_bass_kernel_spmd(nc, in_maps, list(range(N_CORES)))
    _last_results = res

    num = 0.0
    for i in range(N_CORES):
        num += float(res.results[i]["partials"].sum(dtype=np.float64))
    return np.float32(-num / k)
